# revision 9
# baseline (speedup 1.0000x reference)
"""CSPAttention Trainium2 kernel: 8-way SPMD (batch x seq-half), no collectives.

Sharding: core = b*2 + half; each core computes 1024 query rows of batch b
against the full 2048-token K/V of that batch.  Host side does layout
marshalling only (transposes + fp32->bf16 casts); all FLOPs run on device.

Device plan (per core, bf16 matmuls, fp32 PSUM accumulate):
  qkv:    feature-major Q/K (features on partitions, tokens free) via
          weight-stationary matmuls; V token-major into the PV stationary
          layout vaug[kv, kc, h, ones(64)|V(64)] so the softmax denominator
          falls out of the PV matmul broadcast across 64 PSUM partitions.
  attn:   per head, per kv-chunk kc: S.T = K_chunk.T @ Q (2 matmuls into one
          2-bank PSUM tile), one Exp ACTIVATE over [128,1024] (scale=1/8
          folded) -> bf16 SBUF, PV accumulates [denom|O] over kc.  The PE
          instruction stream is software-pipelined (S of chunk kc+1 is
          emitted before PV of chunk kc) so the PE never queue-blocks on the
          ACT exp; conv-branch matmuls are drip-fed into the same loop to
          fill leftover PE slack (the loop is ACT-bound).
  norm:   reciprocal_approx_fast on the denominator rows + DVE multiply.
  tail:   Wo_attn, then Wf token-major + residual + LayerNorm, spread over
          DVE (stats) / ACT (scale-bias apply) / GpSimd (residual prep, out
          DMA with bf16->f32 cast) so no single engine serializes the tail.
"""

import sys

sys.path.insert(0, '/opt/trn_rl_repo')

import numpy as np
import ml_dtypes

import concourse.bass as bass
import concourse.tile as tile
from concourse import bacc, mybir

F32 = mybir.dt.float32
BF16 = mybir.dt.bfloat16
BF = ml_dtypes.bfloat16

B, L, D = 4, 2048, 1024
DA = 512          # attention channels
DC = 512          # conv channels
H = 8             # heads
HD = 64           # head dim
N_CORES = 8
TQ = 1024         # query rows per core
TK = 2048         # kv rows per core
NTQ = TQ // 512   # 512-token tiles of queries
NTK = TK // 512
NQC = TQ // 128   # 128-token query chunks
NKC = TK // 128   # 128-token kv chunks
NDA = DA // 128
ND = D // 128
LN_EPS = 1e-5

Identity = mybir.ActivationFunctionType.Identity
Exp = mybir.ActivationFunctionType.Exp
Sqrt = mybir.ActivationFunctionType.Sqrt
AOp = mybir.AluOpType


def _chunked(t, nch, w, col0=0, ncol=None):
    """DRAM [nch*128, w] -> SBUF-layout AP [128, nch, ncol] starting at col0."""
    if ncol is None:
        ncol = w
    return bass.AP(tensor=t, offset=col0,
                   ap=[[w, 128], [128 * w, nch], [1, ncol]])


def _feat_bias(t, nch):
    """DRAM [nch*128] -> SBUF [128, nch] feature-major bias."""
    return bass.AP(tensor=t, offset=0, ap=[[1, 128], [128, nch]])


def _bcast(t, n):
    """DRAM [n] -> [128, n] partition broadcast."""
    return bass.AP(tensor=t, offset=0, ap=[[0, 128], [1, n]])


def _rows(t, w, r0, nr):
    """DRAM [*, w] rows r0:r0+nr -> SBUF [nr, w]."""
    return bass.AP(tensor=t, offset=r0 * w, ap=[[w, nr], [1, w]])


def build_nc(reps: int = 1):
    nc = bacc.Bacc('TRN2', target_bir_lowering=False, debug=False,
                   num_devices=N_CORES)

    def din(name, shape, dt):
        return nc.dram_tensor(name, list(shape), dt, kind='ExternalInput')

    t = {n: din(n, s, dt) for n, s, dt in [
        ('qaT', [DA, TQ], BF16), ('qcT', [DC, TQ + 2], BF16),
        ('qres', [TQ, D], BF16),
        ('kT', [DA, TK], BF16), ('vT', [DA, TK], BF16),
        ('wqT', [DA, DA], BF16), ('wkT', [DA, DA], BF16),
        ('wvT', [DA, DA], BF16), ('woaT', [DA, DA], BF16),
        ('wocT', [DC, DC], F32), ('wfT', [D, D], BF16),
        ('cw', [DC, 3], F32), ('bq', [DA], F32), ('bk', [DA], F32),
        ('bv', [DA], F32), ('boa', [DA], F32), ('cb', [DC], BF16),
        ('boc', [DC], F32), ('bf', [D], F32), ('gamma', [D], F32),
        ('beta', [D], F32)]}
    t['out'] = nc.dram_tensor('out', [TQ, D], F32, kind='ExternalOutput')

    with tile.TileContext(nc) as tc:
        for rep in range(reps):
            _build_rep(nc, tc, rep, t)
    nc.compile()
    return nc


def _build_rep(nc, tc, rep, t):
    R = f'r{rep}_'
    with tc.tile_pool(name=R + 'persist', bufs=1) as pp:
        fT_s = pp.tile([128, ND, TQ], BF16)       # concat features, fm
        q_s = pp.tile([128, NDA, TQ], BF16)
        k_s = pp.tile([128, NDA, TK], BF16)
        vaug = pp.tile([128, NKC, H, 128], BF16)  # [ones(64) | V(64)]
        o_s = pp.tile([128, NDA, TQ], BF16)       # normalized attn out, fm
        qcT_s = pp.tile([128, NDA, TQ + 2], BF16)
        mc_s = pp.tile([128, 3, NDA, DC], BF16)   # conv folded weights
        cbe_s = pp.tile([128, NDA], F32)          # conv effective bias
        resbf = pp.tile([128, NQC, D], BF16)      # residual + bf, token-major
        bf_bc = pp.tile([128, D], F32)
        wf_s = pp.tile([128, ND, D], BF16)        # prefetched tail weights
        woa_s = pp.tile([128, NDA, DA], BF16)
        boa_s = pp.tile([128, NDA], F32)
        ga_bc = pp.tile([128, D], BF16)
        be_bc = pp.tile([128, D], BF16)
        eps_s = pp.tile([128, 1], F32)
        nc.vector.memset(eps_s, LN_EPS)
        nc.gpsimd.dma_start(out=ga_bc, in_=_bcast(t['gamma'], D))
        nc.gpsimd.dma_start(out=be_bc, in_=_bcast(t['beta'], D))

        # preload the exp table set while DMAs stream
        warm = pp.tile([128, 1], F32)
        nc.vector.memset(warm, 0.0)
        nc.scalar.activation(warm, warm, Exp)

        # ---------------- Q/K/V projections ----------------
        with tc.tile_pool(name=R + 'projw', bufs=1) as wpj, \
             tc.tile_pool(name=R + 'xin', bufs=2) as xp, \
             tc.tile_pool(name=R + 'ps_proj', bufs=2, space='PSUM') as ps_proj:
            wq_s = wpj.tile([128, NDA, DA], BF16, tag='wq')
            nc.sync.dma_start(out=wq_s, in_=_chunked(t['wqT'], NDA, DA))
            bq_s = wpj.tile([128, NDA], F32, tag='bq')
            nc.sync.dma_start(out=bq_s, in_=_feat_bias(t['bq'], NDA))
            nc.vector.memset(vaug[:, :, :, 0:64], 1.0)

            # Q: feature-major out
            for tt in range(NTQ):
                xa = xp.tile([128, NDA, 512], BF16, tag='xa')
                nc.sync.dma_start(
                    out=xa, in_=_chunked(t['qaT'], NDA, TQ, tt * 512, 512))
                for oc in range(NDA):
                    pq = ps_proj.tile([128, 512], F32, tag='projp')
                    for dc in range(NDA):
                        nc.tensor.matmul(
                            pq[:, :],
                            wq_s[:, dc, oc * 128:(oc + 1) * 128],
                            xa[:, dc, :],
                            start=(dc == 0), stop=(dc == NDA - 1))
                    nc.scalar.activation(
                        q_s[:, oc, tt * 512:(tt + 1) * 512], pq[:, :],
                        Identity, bias=bq_s[:, oc:oc + 1])

            # K: feature-major out
            wk_s = wpj.tile([128, NDA, DA], BF16, tag='wk')
            nc.sync.dma_start(out=wk_s, in_=_chunked(t['wkT'], NDA, DA))
            bk_s = wpj.tile([128, NDA], F32, tag='bk')
            nc.sync.dma_start(out=bk_s, in_=_feat_bias(t['bk'], NDA))
            for tt in range(NTK):
                xk = xp.tile([128, NDA, 512], BF16, tag='xk')
                nc.sync.dma_start(
                    out=xk, in_=_chunked(t['kT'], NDA, TK, tt * 512, 512))
                for oc in range(NDA):
                    pk = ps_proj.tile([128, 512], F32, tag='projp')
                    for dc in range(NDA):
                        nc.tensor.matmul(
                            pk[:, :],
                            wk_s[:, dc, oc * 128:(oc + 1) * 128],
                            xk[:, dc, :],
                            start=(dc == 0), stop=(dc == NDA - 1))
                    nc.scalar.activation(
                        k_s[:, oc, tt * 512:(tt + 1) * 512], pk[:, :],
                        Identity, bias=bk_s[:, oc:oc + 1])

            # V: token-major out into vaug[:, kc, h, 64:128]
            wv_s = wpj.tile([128, NDA, DA], BF16, tag='wv')
            nc.sync.dma_start(out=wv_s, in_=_chunked(t['wvT'], NDA, DA))
            bv_bc = wpj.tile([128, DA], F32, tag='bvb')
            nc.sync.dma_start(out=bv_bc, in_=_bcast(t['bv'], DA))
            for tt in range(NTK):
                xv = xp.tile([128, NDA, 512], BF16, tag='xk')
                nc.sync.dma_start(
                    out=xv, in_=_chunked(t['vT'], NDA, TK, tt * 512, 512))
                for kk in range(4):
                    kc = tt * 4 + kk
                    pv = ps_proj.tile([128, 512], F32, tag='projp')
                    for dc in range(NDA):
                        nc.tensor.matmul(
                            pv[:, :],
                            xv[:, dc, kk * 128:(kk + 1) * 128],
                            wv_s[:, dc, :],
                            start=(dc == 0), stop=(dc == NDA - 1))
                    nc.vector.tensor_tensor(
                        vaug[:, kc, :, 64:128],
                        pv[:].rearrange('p (h x) -> p h x', h=H),
                        bv_bc[:].rearrange('p (h x) -> p h x', h=H),
                        AOp.add)

            # conv weight folding + input (DVE/DMA work under the PE qkv)
            nc.sync.dma_start(out=qcT_s, in_=_chunked(t['qcT'], NDA, TQ + 2))
            nc.sync.dma_start(out=bf_bc, in_=_bcast(t['bf'], D))
            # tail weights: prefetch long before use
            nc.sync.dma_start(out=woa_s, in_=_chunked(t['woaT'], NDA, DA))
            nc.sync.dma_start(out=boa_s, in_=_feat_bias(t['boa'], NDA))
            nc.sync.dma_start(out=wf_s, in_=_chunked(t['wfT'], ND, D))
            with tc.tile_pool(name=R + 'convw', bufs=1) as cp, \
                 tc.tile_pool(name=R + 'ps_cb', bufs=1, space='PSUM') as ps_cb:
                wocT_s = cp.tile([128, NDA, DC], F32)
                nc.sync.dma_start(out=wocT_s, in_=_chunked(t['wocT'], NDA, DC))
                wocb_s = cp.tile([128, NDA, DC], BF16)
                nc.vector.tensor_copy(wocb_s, wocT_s)
                cw_s = cp.tile([128, NDA, 3], F32)
                nc.sync.dma_start(out=cw_s, in_=_chunked(t['cw'], NDA, 3))
                cb_s = cp.tile([128, NDA], BF16)
                nc.sync.dma_start(out=cb_s, in_=_feat_bias(t['cb'], NDA))
                boc_s = cp.tile([128, NDA], F32)
                nc.sync.dma_start(out=boc_s, in_=_feat_bias(t['boc'], NDA))

                for k in range(3):
                    for c in range(NDA):
                        nc.vector.tensor_scalar_mul(
                            mc_s[:, k, c, :], wocT_s[:, c, :],
                            cw_s[:, c, k:k + 1])

                # effective conv bias: Wo_conv @ cb + boc
                for oc in range(NDA):
                    pcb = ps_cb.tile([128, 1], F32, tag='cb')
                    for c in range(NDA):
                        nc.tensor.matmul(pcb[:, :],
                                         wocb_s[:, c, oc * 128:(oc + 1) * 128],
                                         cb_s[:, c:c + 1],
                                         start=(c == 0), stop=(c == NDA - 1))
                    nc.scalar.activation(cbe_s[:, oc:oc + 1], pcb[:, :],
                                         Identity, bias=boc_s[:, oc:oc + 1])

        # residual prefetch + bf fold (GpSimd, runs under attention)
        nc.sync.dma_start(out=resbf, in_=_chunked(t['qres'], NQC, D))
        nc.gpsimd.tensor_tensor(
            resbf[:, :, :], resbf[:, :, :],
            bass.AP(tensor=bf_bc.tensor, offset=bf_bc.offset,
                    ap=[[bf_bc.ap[0][0], 128], [0, NQC], [1, D]]),
            AOp.add)

        # ---------------- attention (+ conv matmuls drip-fed) ----------------
        with tc.tile_pool(name=R + 'pst', bufs=2, space='PSUM') as ps_s, \
             tc.tile_pool(name=R + 'pso', bufs=1, space='PSUM') as ps_o, \
             tc.tile_pool(name=R + 'psc', bufs=2, space='PSUM') as ps_c, \
             tc.tile_pool(name=R + 'pwork', bufs=3) as wp, \
             tc.tile_pool(name=R + 'rnorm', bufs=2) as rp:

            # conv matmul drip generator: yields after each emitted matmul
            def conv_steps():
                for oc in range(NDA):
                    for tt in range(NTQ):
                        pc = ps_c.tile([128, 512], F32, tag='convp')
                        n = 0
                        for k in range(3):
                            for c in range(NDA):
                                nc.tensor.matmul(
                                    pc[:, :],
                                    mc_s[:, k, c, oc * 128:(oc + 1) * 128],
                                    qcT_s[:, c, tt * 512 + k: tt * 512 + k + 512],
                                    start=(n == 0), stop=(n == 3 * NDA - 1),
                                    skip_group_check=True)
                                n += 1
                                yield
                        nc.vector.tensor_scalar_add(
                            fT_s[:, NDA + oc, tt * 512:(tt + 1) * 512],
                            pc[:, :], cbe_s[:, oc:oc + 1])

            conv_gen = conv_steps()

            def conv_drip():
                try:
                    next(conv_gen)
                except StopIteration:
                    pass

            for h in range(H):
                hp = (h % 2) * 64
                hc = h // 2
                o_ps = ps_o.tile([128, TQ], F32, tag='o')
                prev = None  # pending PV p_sb
                for kc in range(NKC):
                    s_ps = ps_s.tile([128, TQ], F32, tag='s')
                    for tt in range(NTQ):
                        nc.tensor.matmul(
                            s_ps[:, tt * 512:(tt + 1) * 512],
                            k_s[hp:hp + 64, hc, kc * 128:(kc + 1) * 128],
                            q_s[hp:hp + 64, hc, tt * 512:(tt + 1) * 512],
                            start=True, stop=True)
                    if prev is not None:
                        _emit_pv(nc, o_ps, vaug, prev[1], h, prev[0])
                        conv_drip()
                    p_sb = wp.tile([128, TQ], BF16, tag='p')
                    nc.scalar.activation(p_sb[:, :], s_ps[:, :], Exp,
                                         scale=0.125)
                    prev = (kc, p_sb)
                _emit_pv(nc, o_ps, vaug, prev[1], h, prev[0])
                conv_drip()

                # normalize: rows 0:64 = denominator (broadcast), 64:128 = O
                rec = rp.tile([64, TQ], F32, tag='rec')
                nc.vector.reciprocal_approx_fast(rec[:, :], o_ps[0:64, :])
                nc.vector.tensor_tensor(
                    o_s[hp:hp + 64, hc, :], o_ps[64:128, :], rec[:, :],
                    AOp.mult)

            for _ in range(3 * NDA * NDA * NTQ):
                conv_drip()

        # ---------------- Wo_attn ----------------
        with tc.tile_pool(name=R + 'ps_wo', bufs=2, space='PSUM') as ps_wo:
            for tt in range(NTQ):
                for oc in range(NDA):
                    pa = ps_wo.tile([128, 512], F32, tag='wop')
                    for dc in range(NDA):
                        nc.tensor.matmul(
                            pa[:, :],
                            woa_s[:, dc, oc * 128:(oc + 1) * 128],
                            o_s[:, dc, tt * 512:(tt + 1) * 512],
                            start=(dc == 0), stop=(dc == NDA - 1))
                    nc.scalar.activation(
                        fT_s[:, oc, tt * 512:(tt + 1) * 512], pa[:, :],
                        Identity, bias=boa_s[:, oc:oc + 1])

        # ---------------- Wf + residual + LayerNorm ----------------
        with tc.tile_pool(name=R + 'lnw', bufs=3) as lp, \
             tc.tile_pool(name=R + 'ps_f', bufs=2, space='PSUM') as ps_f:
            for i in range(NQC):
                pf = ps_f.tile([128, D], F32, tag='f')
                for ot in range(2):
                    for fc in range(ND):
                        nc.tensor.matmul(
                            pf[:, ot * 512:(ot + 1) * 512],
                            fT_s[:, fc, i * 128:(i + 1) * 128],
                            wf_s[:, fc, ot * 512:(ot + 1) * 512],
                            start=(fc == 0), stop=(fc == ND - 1))
                x_s = lp.tile([128, D], F32, tag='x')
                nc.vector.scalar_tensor_tensor(x_s[:, :], pf[:, :], 1.0,
                                               resbf[:, i, :], AOp.mult,
                                               AOp.add)
                stats = lp.tile([128, 2, 6], F32, tag='st')
                nc.vector.bn_stats(stats[:, 0, :], x_s[:, 0:512])
                nc.vector.bn_stats(stats[:, 1, :], x_s[:, 512:1024])
                mv = lp.tile([128, 2], F32, tag='mv')
                nc.vector.bn_aggr(mv[:, :], stats[:, :, :])
                sd = lp.tile([128, 1], F32, tag='sd')
                nc.scalar.activation(sd[:, :], mv[:, 1:2], Sqrt,
                                     bias=eps_s[:, 0:1])
                rstd = lp.tile([128, 1], F32, tag='rs')
                nc.vector.reciprocal(rstd[:, :], sd[:, :])
                nm = lp.tile([128, 1], F32, tag='nm')
                nc.vector.scalar_tensor_tensor(nm[:, :], mv[:, 0:1], -1.0,
                                               rstd[:, :], AOp.mult, AOp.mult)
                t1 = lp.tile([128, D], BF16, tag='t1')
                nc.scalar.activation(t1[:, :], x_s[:, :], Identity,
                                     bias=nm[:, 0:1], scale=rstd[:, 0:1])
                o1 = lp.tile([128, D], BF16, tag='o1')
                nc.vector.tensor_tensor(o1[:, :], t1[:, :], ga_bc[:, :],
                                        AOp.mult)
                o_sb = lp.tile([128, D], BF16, tag='ob')
                nc.gpsimd.tensor_tensor(o_sb[:, :], o1[:, :], be_bc[:, :],
                                        AOp.add)
                nc.gpsimd.dma_start(out=_rows(t['out'], D, i * 128, 128),
                                    in_=o_sb[:, :])


def _emit_pv(nc, o_ps, vaug, p_sb, h, kc):
    for tt in range(NTQ):
        nc.tensor.matmul(
            o_ps[:, tt * 512:(tt + 1) * 512],
            vaug[:, kc, h, :],
            p_sb[:, tt * 512:(tt + 1) * 512],
            start=(kc == 0), stop=(kc == NKC - 1),
            skip_group_check=True)


def make_in_maps(inputs):
    q = np.ascontiguousarray(np.asarray(inputs['queries'], np.float32))
    k = np.ascontiguousarray(np.asarray(inputs['keys'], np.float32))
    v = np.ascontiguousarray(np.asarray(inputs['values'], np.float32))
    W = {n: np.ascontiguousarray(np.asarray(inputs[n], np.float32).T)
         for n in ('Wq', 'Wk', 'Wv', 'Wo_attn', 'Wo_conv', 'Wf')}
    com = {
        'wqT': W['Wq'].astype(BF), 'wkT': W['Wk'].astype(BF),
        'wvT': W['Wv'].astype(BF), 'woaT': W['Wo_attn'].astype(BF),
        'wocT': W['Wo_conv'], 'wfT': W['Wf'].astype(BF),
        'cw': np.asarray(inputs['conv_w'], np.float32).reshape(DC, 3),
        'bq': np.asarray(inputs['bq'], np.float32),
        'bk': np.asarray(inputs['bk'], np.float32),
        'bv': np.asarray(inputs['bv'], np.float32),
        'boa': np.asarray(inputs['bo_attn'], np.float32),
        'cb': np.asarray(inputs['conv_b'], np.float32).astype(BF),
        'boc': np.asarray(inputs['bo_conv'], np.float32),
        'bf': np.asarray(inputs['bf'], np.float32),
        'gamma': np.asarray(inputs['gamma'], np.float32),
        'beta': np.asarray(inputs['beta'], np.float32),
    }
    com = {n: np.ascontiguousarray(a) for n, a in com.items()}
    in_maps = []
    for core in range(N_CORES):
        b, half = core // 2, core % 2
        r0, r1 = half * TQ, (half + 1) * TQ
        qc = np.zeros((TQ + 2, DC), np.float32)
        qc[1:TQ + 1] = q[b, r0:r1, DA:]
        if r0 > 0:
            qc[0] = q[b, r0 - 1, DA:]
        if r1 < L:
            qc[TQ + 1] = q[b, r1, DA:]
        m = dict(com)
        m['qaT'] = np.ascontiguousarray(q[b, r0:r1, :DA].T).astype(BF)
        m['qcT'] = np.ascontiguousarray(qc.T).astype(BF)
        m['qres'] = np.ascontiguousarray(q[b, r0:r1, :]).astype(BF)
        m['kT'] = np.ascontiguousarray(k[b, :, :DA].T).astype(BF)
        m['vT'] = np.ascontiguousarray(v[b, :, :DA].T).astype(BF)
        in_maps.append(m)
    return in_maps


_NC_CACHE = {}


def get_nc(reps=1):
    if reps not in _NC_CACHE:
        _NC_CACHE[reps] = build_nc(reps)
    return _NC_CACHE[reps]


def kernel(**inputs):
    from concourse.bass_utils import run_bass_kernel_spmd
    nc = get_nc(1)
    in_maps = make_in_maps(inputs)
    res = run_bass_kernel_spmd(nc, in_maps, core_ids=list(range(N_CORES)))
    out = np.empty((B, L, D), np.float32)
    for core in range(N_CORES):
        b, half = core // 2, core % 2
        out[b, half * TQ:(half + 1) * TQ, :] = res.results[core]['out']
    return out


# revision 12
# speedup vs baseline: 1.0122x; 1.0122x over previous
"""CSPAttention Trainium2 kernel: 8-way SPMD (batch x seq-half), no collectives.

Sharding: core = b*2 + half; each core computes 1024 query rows of batch b
against the full 2048-token K/V of that batch.  Host side does layout
marshalling only (transposes + fp32->bf16 casts); all FLOPs run on device.

Device plan (per core, bf16 matmuls, fp32 PSUM accumulate):
  qkv:    feature-major Q/K (features on partitions, tokens free) via
          weight-stationary matmuls; V token-major into the PV stationary
          layout vaug[kv, kc, h, ones(64)|V(64)] so the softmax denominator
          falls out of the PV matmul broadcast across 64 PSUM partitions.
  attn:   per head, per kv-chunk kc: S.T = K_chunk.T @ Q (2 matmuls into one
          2-bank PSUM tile), one Exp ACTIVATE over [128,1024] (scale=1/8
          folded) -> bf16 SBUF, PV accumulates [denom|O] over kc.  The PE
          instruction stream is software-pipelined (S of chunk kc+1 is
          emitted before PV of chunk kc) so the PE never queue-blocks on the
          ACT exp; conv-branch matmuls are drip-fed into the same loop to
          fill leftover PE slack (the loop is ACT-bound).
  norm:   reciprocal_approx_fast on the denominator rows + DVE multiply.
  tail:   Wo_attn, then Wf token-major + residual + LayerNorm, spread over
          DVE (stats) / ACT (scale-bias apply) / GpSimd (residual prep, out
          DMA with bf16->f32 cast) so no single engine serializes the tail.
"""

import sys

sys.path.insert(0, '/opt/trn_rl_repo')

import numpy as np
import ml_dtypes

import concourse.bass as bass
import concourse.tile as tile
from concourse import bacc, mybir

F32 = mybir.dt.float32
BF16 = mybir.dt.bfloat16
BF = ml_dtypes.bfloat16

B, L, D = 4, 2048, 1024
DA = 512          # attention channels
DC = 512          # conv channels
H = 8             # heads
HD = 64           # head dim
N_CORES = 8
TQ = 1024         # query rows per core
TK = 2048         # kv rows per core
NTQ = TQ // 512   # 512-token tiles of queries
NTK = TK // 512
NQC = TQ // 128   # 128-token query chunks
NKC = TK // 128   # 128-token kv chunks
NDA = DA // 128
ND = D // 128
LN_EPS = 1e-5

Identity = mybir.ActivationFunctionType.Identity
Exp = mybir.ActivationFunctionType.Exp
Sqrt = mybir.ActivationFunctionType.Sqrt
AOp = mybir.AluOpType


def _chunked(t, nch, w, col0=0, ncol=None):
    """DRAM [nch*128, w] -> SBUF-layout AP [128, nch, ncol] starting at col0."""
    if ncol is None:
        ncol = w
    return bass.AP(tensor=t, offset=col0,
                   ap=[[w, 128], [128 * w, nch], [1, ncol]])


def _feat_bias(t, nch):
    """DRAM [nch*128] -> SBUF [128, nch] feature-major bias."""
    return bass.AP(tensor=t, offset=0, ap=[[1, 128], [128, nch]])


def _bcast(t, n):
    """DRAM [n] -> [128, n] partition broadcast."""
    return bass.AP(tensor=t, offset=0, ap=[[0, 128], [1, n]])


def _rows(t, w, r0, nr):
    """DRAM [*, w] rows r0:r0+nr -> SBUF [nr, w]."""
    return bass.AP(tensor=t, offset=r0 * w, ap=[[w, nr], [1, w]])


def build_nc(reps: int = 1):
    nc = bacc.Bacc('TRN2', target_bir_lowering=False, debug=False,
                   num_devices=N_CORES)

    def din(name, shape, dt):
        return nc.dram_tensor(name, list(shape), dt, kind='ExternalInput')

    t = {n: din(n, s, dt) for n, s, dt in [
        ('qaT', [DA, TQ], BF16), ('qcT', [DC, TQ + 2], BF16),
        ('qres', [TQ, D], BF16),
        ('kT', [DA, TK], BF16), ('vT', [DA, TK], BF16),
        ('wqT', [DA, DA], BF16), ('wkT', [DA, DA], BF16),
        ('wvT', [DA, DA], BF16), ('woaT', [DA, DA], BF16),
        ('wocT', [DC, DC], F32), ('wfT', [D, D], BF16),
        ('cw', [DC, 3], F32), ('bq', [DA], F32), ('bk', [DA], F32),
        ('bv', [DA], F32), ('boa', [DA], F32), ('cb', [DC], F32),
        ('boc', [DC], F32), ('bf', [D], F32), ('gamma', [D], F32),
        ('beta', [D], F32)]}
    t['out'] = nc.dram_tensor('out', [TQ, D], F32, kind='ExternalOutput')

    with tile.TileContext(nc) as tc:
        for rep in range(reps):
            _build_rep(nc, tc, rep, t)
    nc.compile()
    return nc


def _build_rep(nc, tc, rep, t):
    R = f'r{rep}_'
    with tc.tile_pool(name=R + 'persist', bufs=1) as pp:
        fT_s = pp.tile([128, ND, TQ], BF16)       # concat features, fm
        q_s = pp.tile([128, NDA, TQ], BF16)
        k_s = pp.tile([128, NDA, TK], BF16)
        vaug = pp.tile([128, NKC, H, 128], BF16)  # [ones(64) | V(64)]
        o_s = pp.tile([128, NDA, TQ], BF16)       # normalized attn out, fm
        qcT_s = pp.tile([128, NDA, TQ + 2], BF16)
        mc_s = pp.tile([128, 3, NDA, DC], BF16)   # conv folded weights
        cbe_s = pp.tile([128, NDA], F32)          # conv effective bias
        resbf = pp.tile([128, NQC, D], BF16)      # residual + bf, token-major
        bf_bc = pp.tile([128, D], F32)
        wf_s = pp.tile([128, ND, D], BF16)        # prefetched tail weights
        woa_s = pp.tile([128, NDA, DA], BF16)
        boa_s = pp.tile([128, NDA], F32)
        ga_bc = pp.tile([128, D], BF16)
        be_bc = pp.tile([128, D], BF16)
        eps_s = pp.tile([128, 1], F32)
        nc.vector.memset(eps_s, LN_EPS)
        nc.gpsimd.dma_start(out=ga_bc, in_=_bcast(t['gamma'], D))
        nc.gpsimd.dma_start(out=be_bc, in_=_bcast(t['beta'], D))

        # preload the exp table set while DMAs stream
        warm = pp.tile([128, 1], F32)
        nc.vector.memset(warm, 0.0)
        nc.scalar.activation(warm, warm, Exp)

        # ---------------- Q/K/V projections ----------------
        with tc.tile_pool(name=R + 'projw', bufs=1) as wpj, \
             tc.tile_pool(name=R + 'inp', bufs=1) as ip, \
             tc.tile_pool(name=R + 'ps_proj', bufs=2, space='PSUM') as ps_proj:
            wq_s = wpj.tile([128, NDA, DA], BF16, tag='wq')
            nc.sync.dma_start(out=wq_s, in_=_chunked(t['wqT'], NDA, DA))
            bq_s = wpj.tile([128, NDA], F32, tag='bq')
            nc.sync.dma_start(out=bq_s, in_=_feat_bias(t['bq'], NDA))
            qa_in = ip.tile([128, NDA, TQ], BF16, tag='qa')
            nc.sync.dma_start(out=qa_in, in_=_chunked(t['qaT'], NDA, TQ))
            wk_s = wpj.tile([128, NDA, DA], BF16, tag='wk')
            nc.sync.dma_start(out=wk_s, in_=_chunked(t['wkT'], NDA, DA))
            bk_s = wpj.tile([128, NDA], F32, tag='bk')
            nc.sync.dma_start(out=bk_s, in_=_feat_bias(t['bk'], NDA))
            k_in = ip.tile([128, NDA, TK], BF16, tag='ki')
            nc.sync.dma_start(out=k_in, in_=_chunked(t['kT'], NDA, TK))
            wv_s = wpj.tile([128, NDA, DA], BF16, tag='wv')
            nc.sync.dma_start(out=wv_s, in_=_chunked(t['wvT'], NDA, DA))
            bv_bc = wpj.tile([128, DA], F32, tag='bvb')
            nc.sync.dma_start(out=bv_bc, in_=_bcast(t['bv'], DA))
            v_in = ip.tile([128, NDA, TK], BF16, tag='vi')
            nc.sync.dma_start(out=v_in, in_=_chunked(t['vT'], NDA, TK))
            nc.sync.dma_start(out=qcT_s, in_=_chunked(t['qcT'], NDA, TQ + 2))
            nc.sync.dma_start(out=bf_bc, in_=_bcast(t['bf'], D))
            # tail weights on the ACT HWDGE queue (separate FIFO)
            nc.scalar.dma_start(out=woa_s, in_=_chunked(t['woaT'], NDA, DA))
            nc.scalar.dma_start(out=boa_s, in_=_feat_bias(t['boa'], NDA))
            nc.scalar.dma_start(out=wf_s, in_=_chunked(t['wfT'], ND, D))
            nc.vector.memset(vaug[:, :, :, 0:64], 1.0)

            # Q: feature-major out
            for tt in range(NTQ):
                for oc in range(NDA):
                    pq = ps_proj.tile([128, 512], F32, tag='projp')
                    for dc in range(NDA):
                        nc.tensor.matmul(
                            pq[:, :],
                            wq_s[:, dc, oc * 128:(oc + 1) * 128],
                            qa_in[:, dc, tt * 512:(tt + 1) * 512],
                            start=(dc == 0), stop=(dc == NDA - 1))
                    nc.scalar.activation(
                        q_s[:, oc, tt * 512:(tt + 1) * 512], pq[:, :],
                        Identity, bias=bq_s[:, oc:oc + 1])

            # K: feature-major out
            for tt in range(NTK):
                for oc in range(NDA):
                    pk = ps_proj.tile([128, 512], F32, tag='projp')
                    for dc in range(NDA):
                        nc.tensor.matmul(
                            pk[:, :],
                            wk_s[:, dc, oc * 128:(oc + 1) * 128],
                            k_in[:, dc, tt * 512:(tt + 1) * 512],
                            start=(dc == 0), stop=(dc == NDA - 1))
                    nc.scalar.activation(
                        k_s[:, oc, tt * 512:(tt + 1) * 512], pk[:, :],
                        Identity, bias=bk_s[:, oc:oc + 1])

            # V: token-major out into vaug[:, kc, h, 64:128]
            for kc in range(NKC):
                pv = ps_proj.tile([128, 512], F32, tag='projp')
                for dc in range(NDA):
                    nc.tensor.matmul(
                        pv[:, :],
                        v_in[:, dc, kc * 128:(kc + 1) * 128],
                        wv_s[:, dc, :],
                        start=(dc == 0), stop=(dc == NDA - 1))
                nc.vector.tensor_tensor(
                    vaug[:, kc, :, 64:128],
                    pv[:].rearrange('p (h x) -> p h x', h=H),
                    bv_bc[:].rearrange('p (h x) -> p h x', h=H),
                    AOp.add)
        # conv weight folding (runs at attention start; inp tiles freed above)
        with tc.tile_pool(name=R + 'convw', bufs=1) as cp, \
             tc.tile_pool(name=R + 'ps_cb', bufs=1, space='PSUM') as ps_cb:
            wocT_s = cp.tile([128, NDA, DC], F32)
            nc.sync.dma_start(out=wocT_s, in_=_chunked(t['wocT'], NDA, DC))
            cw_s = cp.tile([128, NDA, 3], F32)
            nc.sync.dma_start(out=cw_s, in_=_chunked(t['cw'], NDA, 3))
            cb_s = cp.tile([128, NDA], F32)
            nc.sync.dma_start(out=cb_s, in_=_feat_bias(t['cb'], NDA))
            boc_s = cp.tile([128, NDA], F32)
            nc.sync.dma_start(out=boc_s, in_=_feat_bias(t['boc'], NDA))

            for k in range(3):
                for c in range(NDA):
                    nc.vector.tensor_scalar_mul(
                        mc_s[:, k, c, :], wocT_s[:, c, :],
                        cw_s[:, c, k:k + 1])

            # effective conv bias: Wo_conv @ cb + boc
            for oc in range(NDA):
                pcb = ps_cb.tile([128, 1], F32, tag='cb')
                for c in range(NDA):
                    nc.tensor.matmul(pcb[:, :],
                                     wocT_s[:, c, oc * 128:(oc + 1) * 128],
                                     cb_s[:, c:c + 1],
                                     start=(c == 0), stop=(c == NDA - 1))
                nc.scalar.activation(cbe_s[:, oc:oc + 1], pcb[:, :],
                                     Identity, bias=boc_s[:, oc:oc + 1])

        # residual prefetch + bf fold (GpSimd, runs under attention)
        nc.sync.dma_start(out=resbf, in_=_chunked(t['qres'], NQC, D))
        nc.gpsimd.tensor_tensor(
            resbf[:, :, :], resbf[:, :, :],
            bass.AP(tensor=bf_bc.tensor, offset=bf_bc.offset,
                    ap=[[bf_bc.ap[0][0], 128], [0, NQC], [1, D]]),
            AOp.add)

        # ---------------- attention (+ conv matmuls drip-fed) ----------------
        with tc.tile_pool(name=R + 'pst', bufs=2, space='PSUM') as ps_s, \
             tc.tile_pool(name=R + 'pso', bufs=1, space='PSUM') as ps_o, \
             tc.tile_pool(name=R + 'psc', bufs=2, space='PSUM') as ps_c, \
             tc.tile_pool(name=R + 'pwork', bufs=3) as wp, \
             tc.tile_pool(name=R + 'rnorm', bufs=2) as rp:

            # conv matmul drip generator: yields after each emitted matmul
            def conv_steps():
                for oc in range(NDA):
                    for tt in range(NTQ):
                        pc = ps_c.tile([128, 512], F32, tag='convp')
                        n = 0
                        for k in range(3):
                            for c in range(NDA):
                                nc.tensor.matmul(
                                    pc[:, :],
                                    mc_s[:, k, c, oc * 128:(oc + 1) * 128],
                                    qcT_s[:, c, tt * 512 + k: tt * 512 + k + 512],
                                    start=(n == 0), stop=(n == 3 * NDA - 1),
                                    skip_group_check=True)
                                n += 1
                                yield
                        nc.vector.tensor_scalar_add(
                            fT_s[:, NDA + oc, tt * 512:(tt + 1) * 512],
                            pc[:, :], cbe_s[:, oc:oc + 1])

            conv_gen = conv_steps()

            def conv_drip():
                try:
                    next(conv_gen)
                except StopIteration:
                    pass

            for h in range(H):
                hp = (h % 2) * 64
                hc = h // 2
                o_ps = ps_o.tile([128, TQ], F32, tag='o')
                prev = None  # pending PV p_sb
                for kc in range(NKC):
                    s_ps = ps_s.tile([128, TQ], F32, tag='s')
                    for tt in range(NTQ):
                        nc.tensor.matmul(
                            s_ps[:, tt * 512:(tt + 1) * 512],
                            k_s[hp:hp + 64, hc, kc * 128:(kc + 1) * 128],
                            q_s[hp:hp + 64, hc, tt * 512:(tt + 1) * 512],
                            start=True, stop=True)
                    if prev is not None:
                        _emit_pv(nc, o_ps, vaug, prev[1], h, prev[0])
                        conv_drip()
                    p_sb = wp.tile([128, TQ], BF16, tag='p')
                    nc.scalar.activation(p_sb[:, :], s_ps[:, :], Exp,
                                         scale=0.125)
                    prev = (kc, p_sb)
                _emit_pv(nc, o_ps, vaug, prev[1], h, prev[0])
                conv_drip()

                # normalize: rows 0:64 = denominator (broadcast), 64:128 = O
                rec = rp.tile([64, TQ], F32, tag='rec')
                nc.vector.reciprocal_approx_fast(rec[:, :], o_ps[0:64, :])
                nc.vector.tensor_tensor(
                    o_s[hp:hp + 64, hc, :], o_ps[64:128, :], rec[:, :],
                    AOp.mult)

            for _ in range(3 * NDA * NDA * NTQ):
                conv_drip()

        # ---------------- Wo_attn ----------------
        with tc.tile_pool(name=R + 'ps_wo', bufs=2, space='PSUM') as ps_wo:
            for tt in range(NTQ):
                for oc in range(NDA):
                    pa = ps_wo.tile([128, 512], F32, tag='wop')
                    for dc in range(NDA):
                        nc.tensor.matmul(
                            pa[:, :],
                            woa_s[:, dc, oc * 128:(oc + 1) * 128],
                            o_s[:, dc, tt * 512:(tt + 1) * 512],
                            start=(dc == 0), stop=(dc == NDA - 1))
                    nc.scalar.activation(
                        fT_s[:, oc, tt * 512:(tt + 1) * 512], pa[:, :],
                        Identity, bias=boa_s[:, oc:oc + 1])

        # ---------------- Wf + residual + LayerNorm ----------------
        with tc.tile_pool(name=R + 'lnw', bufs=3) as lp, \
             tc.tile_pool(name=R + 'ps_f', bufs=2, space='PSUM') as ps_f:
            for i in range(NQC):
                pf = ps_f.tile([128, D], F32, tag='f')
                for ot in range(2):
                    for fc in range(ND):
                        nc.tensor.matmul(
                            pf[:, ot * 512:(ot + 1) * 512],
                            fT_s[:, fc, i * 128:(i + 1) * 128],
                            wf_s[:, fc, ot * 512:(ot + 1) * 512],
                            start=(fc == 0), stop=(fc == ND - 1))
                x_s = lp.tile([128, D], F32, tag='x')
                nc.vector.scalar_tensor_tensor(x_s[:, :], pf[:, :], 1.0,
                                               resbf[:, i, :], AOp.mult,
                                               AOp.add)
                stats = lp.tile([128, 2, 6], F32, tag='st')
                nc.vector.bn_stats(stats[:, 0, :], x_s[:, 0:512])
                nc.vector.bn_stats(stats[:, 1, :], x_s[:, 512:1024])
                mv = lp.tile([128, 2], F32, tag='mv')
                nc.vector.bn_aggr(mv[:, :], stats[:, :, :])
                sd = lp.tile([128, 1], F32, tag='sd')
                nc.scalar.activation(sd[:, :], mv[:, 1:2], Sqrt,
                                     bias=eps_s[:, 0:1])
                rstd = lp.tile([128, 1], F32, tag='rs')
                nc.vector.reciprocal(rstd[:, :], sd[:, :])
                nm = lp.tile([128, 1], F32, tag='nm')
                nc.vector.scalar_tensor_tensor(nm[:, :], mv[:, 0:1], -1.0,
                                               rstd[:, :], AOp.mult, AOp.mult)
                t1 = lp.tile([128, D], BF16, tag='t1')
                nc.scalar.activation(t1[:, :], x_s[:, :], Identity,
                                     bias=nm[:, 0:1], scale=rstd[:, 0:1])
                o1 = lp.tile([128, D], BF16, tag='o1')
                nc.vector.tensor_tensor(o1[:, :], t1[:, :], ga_bc[:, :],
                                        AOp.mult)
                o_sb = lp.tile([128, D], BF16, tag='ob')
                nc.gpsimd.tensor_tensor(o_sb[:, :], o1[:, :], be_bc[:, :],
                                        AOp.add)
                nc.gpsimd.dma_start(out=_rows(t['out'], D, i * 128, 128),
                                    in_=o_sb[:, :])


def _emit_pv(nc, o_ps, vaug, p_sb, h, kc):
    for tt in range(NTQ):
        nc.tensor.matmul(
            o_ps[:, tt * 512:(tt + 1) * 512],
            vaug[:, kc, h, :],
            p_sb[:, tt * 512:(tt + 1) * 512],
            start=(kc == 0), stop=(kc == NKC - 1),
            skip_group_check=True)


def make_in_maps(inputs):
    q = np.ascontiguousarray(np.asarray(inputs['queries'], np.float32))
    k = np.ascontiguousarray(np.asarray(inputs['keys'], np.float32))
    v = np.ascontiguousarray(np.asarray(inputs['values'], np.float32))
    W = {n: np.ascontiguousarray(np.asarray(inputs[n], np.float32).T)
         for n in ('Wq', 'Wk', 'Wv', 'Wo_attn', 'Wo_conv', 'Wf')}
    com = {
        'wqT': W['Wq'].astype(BF), 'wkT': W['Wk'].astype(BF),
        'wvT': W['Wv'].astype(BF), 'woaT': W['Wo_attn'].astype(BF),
        'wocT': W['Wo_conv'], 'wfT': W['Wf'].astype(BF),
        'cw': np.asarray(inputs['conv_w'], np.float32).reshape(DC, 3),
        'bq': np.asarray(inputs['bq'], np.float32),
        'bk': np.asarray(inputs['bk'], np.float32),
        'bv': np.asarray(inputs['bv'], np.float32),
        'boa': np.asarray(inputs['bo_attn'], np.float32),
        'cb': np.asarray(inputs['conv_b'], np.float32),
        'boc': np.asarray(inputs['bo_conv'], np.float32),
        'bf': np.asarray(inputs['bf'], np.float32),
        'gamma': np.asarray(inputs['gamma'], np.float32),
        'beta': np.asarray(inputs['beta'], np.float32),
    }
    com = {n: np.ascontiguousarray(a) for n, a in com.items()}
    in_maps = []
    for core in range(N_CORES):
        b, half = core // 2, core % 2
        r0, r1 = half * TQ, (half + 1) * TQ
        qc = np.zeros((TQ + 2, DC), np.float32)
        qc[1:TQ + 1] = q[b, r0:r1, DA:]
        if r0 > 0:
            qc[0] = q[b, r0 - 1, DA:]
        if r1 < L:
            qc[TQ + 1] = q[b, r1, DA:]
        m = dict(com)
        m['qaT'] = np.ascontiguousarray(q[b, r0:r1, :DA].T).astype(BF)
        m['qcT'] = np.ascontiguousarray(qc.T).astype(BF)
        m['qres'] = np.ascontiguousarray(q[b, r0:r1, :]).astype(BF)
        m['kT'] = np.ascontiguousarray(k[b, :, :DA].T).astype(BF)
        m['vT'] = np.ascontiguousarray(v[b, :, :DA].T).astype(BF)
        in_maps.append(m)
    return in_maps


_NC_CACHE = {}


def get_nc(reps=1):
    if reps not in _NC_CACHE:
        _NC_CACHE[reps] = build_nc(reps)
    return _NC_CACHE[reps]


def kernel(**inputs):
    from concourse.bass_utils import run_bass_kernel_spmd
    nc = get_nc(1)
    in_maps = make_in_maps(inputs)
    res = run_bass_kernel_spmd(nc, in_maps, core_ids=list(range(N_CORES)))
    out = np.empty((B, L, D), np.float32)
    for core in range(N_CORES):
        b, half = core // 2, core % 2
        out[b, half * TQ:(half + 1) * TQ, :] = res.results[core]['out']
    return out


# revision 13
# speedup vs baseline: 1.0558x; 1.0430x over previous
"""CSPAttention Trainium2 kernel: 8-way SPMD (batch x seq-half), no collectives.

Sharding: core = b*2 + half; each core computes 1024 query rows of batch b
against the full 2048-token K/V of that batch.  Host side does layout
marshalling only (transposes + fp32->bf16 casts); all FLOPs run on device.

Device plan (per core, bf16 matmuls, fp32 PSUM accumulate):
  qkv:    feature-major Q/K (features on partitions, tokens free) via
          weight-stationary matmuls; V token-major into the PV stationary
          layout vaug[kv, kc, h, ones(64)|V(64)] so the softmax denominator
          falls out of the PV matmul broadcast across 64 PSUM partitions.
  attn:   per head, per kv-chunk kc: S.T = K_chunk.T @ Q (2 matmuls into one
          2-bank PSUM tile), one Exp ACTIVATE over [128,1024] (scale=1/8
          folded) -> bf16 SBUF, PV accumulates [denom|O] over kc.  The PE
          instruction stream is software-pipelined (S of chunk kc+1 is
          emitted before PV of chunk kc) so the PE never queue-blocks on the
          ACT exp; conv-branch matmuls are drip-fed into the same loop to
          fill leftover PE slack (the loop is ACT-bound).
  norm:   reciprocal_approx_fast on the denominator rows + DVE multiply.
  tail:   Wo_attn, then Wf token-major + residual + LayerNorm, spread over
          DVE (stats) / ACT (scale-bias apply) / GpSimd (residual prep, out
          DMA with bf16->f32 cast) so no single engine serializes the tail.
"""

import sys

sys.path.insert(0, '/opt/trn_rl_repo')

import numpy as np
import ml_dtypes

import concourse.bass as bass
import concourse.tile as tile
from concourse import bacc, mybir

F32 = mybir.dt.float32
BF16 = mybir.dt.bfloat16
BF = ml_dtypes.bfloat16

B, L, D = 4, 2048, 1024
DA = 512          # attention channels
DC = 512          # conv channels
H = 8             # heads
HD = 64           # head dim
N_CORES = 8
TQ = 1024         # query rows per core
TK = 2048         # kv rows per core
NTQ = TQ // 512   # 512-token tiles of queries
NTK = TK // 512
NQC = TQ // 128   # 128-token query chunks
NKC = TK // 128   # 128-token kv chunks
NDA = DA // 128
ND = D // 128
LN_EPS = 1e-5

Identity = mybir.ActivationFunctionType.Identity
Exp = mybir.ActivationFunctionType.Exp
Sqrt = mybir.ActivationFunctionType.Sqrt
AOp = mybir.AluOpType


def _chunked(t, nch, w, col0=0, ncol=None):
    """DRAM [nch*128, w] -> SBUF-layout AP [128, nch, ncol] starting at col0."""
    if ncol is None:
        ncol = w
    return bass.AP(tensor=t, offset=col0,
                   ap=[[w, 128], [128 * w, nch], [1, ncol]])


def _feat_bias(t, nch):
    """DRAM [nch*128] -> SBUF [128, nch] feature-major bias."""
    return bass.AP(tensor=t, offset=0, ap=[[1, 128], [128, nch]])


def _bcast(t, n):
    """DRAM [n] -> [128, n] partition broadcast."""
    return bass.AP(tensor=t, offset=0, ap=[[0, 128], [1, n]])


def _rows(t, w, r0, nr):
    """DRAM [*, w] rows r0:r0+nr -> SBUF [nr, w]."""
    return bass.AP(tensor=t, offset=r0 * w, ap=[[w, nr], [1, w]])


def build_nc(reps: int = 1):
    nc = bacc.Bacc('TRN2', target_bir_lowering=False, debug=False,
                   num_devices=N_CORES)

    def din(name, shape, dt):
        return nc.dram_tensor(name, list(shape), dt, kind='ExternalInput')

    t = {n: din(n, s, dt) for n, s, dt in [
        ('qaT', [DA, TQ], BF16), ('qcT', [DC, TQ + 2], BF16),
        ('qres', [TQ, D], BF16),
        ('kT', [DA, TK], BF16), ('vT', [DA, TK], BF16),
        ('wqT', [DA, DA], BF16), ('wkT', [DA, DA], BF16),
        ('wvT', [DA, DA], BF16), ('woaT', [DA, DA], BF16),
        ('wocT', [DC, DC], F32), ('wfT', [D, D], BF16),
        ('cw', [DC, 3], F32), ('bq', [DA], F32), ('bk', [DA], F32),
        ('bv', [DA], F32), ('boa', [DA], F32), ('cb', [DC], F32),
        ('boc', [DC], F32), ('bf', [D], F32), ('gamma', [D], F32),
        ('beta', [D], F32)]}
    t['out'] = nc.dram_tensor('out', [TQ, D], F32, kind='ExternalOutput')

    with tile.TileContext(nc) as tc:
        for rep in range(reps):
            _build_rep(nc, tc, rep, t)
    nc.compile()
    return nc


def _build_rep(nc, tc, rep, t):
    R = f'r{rep}_'
    with tc.tile_pool(name=R + 'persist', bufs=1) as pp:
        fT_s = pp.tile([128, ND, TQ], BF16)       # concat features, fm
        q_s = pp.tile([128, NDA, TQ], BF16)
        k_s = pp.tile([128, NDA, TK], BF16)
        vaug = pp.tile([128, NKC, H, 128], BF16)  # [ones(64) | V(64)]
        o_s = pp.tile([128, NDA, TQ], BF16)       # normalized attn out, fm
        qcT_s = pp.tile([128, NDA, TQ + 2], BF16)
        mc_s = pp.tile([128, 3, NDA, DC], BF16)   # conv folded weights
        cbe_s = pp.tile([128, NDA], F32)          # conv effective bias
        resbf = pp.tile([128, NQC, D], BF16)      # residual + bf, token-major
        bf_bc = pp.tile([128, D], F32)
        wf_s = pp.tile([128, ND, D], BF16)        # prefetched tail weights
        woa_s = pp.tile([128, NDA, DA], BF16)
        boa_s = pp.tile([128, NDA], F32)
        ga_bc = pp.tile([128, D], BF16)
        be_bc = pp.tile([128, D], BF16)
        eps_s = pp.tile([128, 1], F32)
        nc.vector.memset(eps_s, LN_EPS)
        nc.gpsimd.dma_start(out=ga_bc, in_=_bcast(t['gamma'], D))
        nc.gpsimd.dma_start(out=be_bc, in_=_bcast(t['beta'], D))

        # preload the exp table set while DMAs stream
        warm = pp.tile([128, 1], F32)
        nc.vector.memset(warm, 0.0)
        nc.scalar.activation(warm, warm, Exp)

        # ---------------- conv weight folding (ACT-queue DMAs) ------------
        with tc.tile_pool(name=R + 'convw', bufs=1) as cp, \
             tc.tile_pool(name=R + 'ps_cb', bufs=1, space='PSUM') as ps_cb:
            wocT_s = cp.tile([128, NDA, DC], F32)
            nc.scalar.dma_start(out=wocT_s, in_=_chunked(t['wocT'], NDA, DC))
            cw_s = cp.tile([128, NDA, 3], F32)
            nc.scalar.dma_start(out=cw_s, in_=_chunked(t['cw'], NDA, 3))
            cb_s = cp.tile([128, NDA], F32)
            nc.scalar.dma_start(out=cb_s, in_=_feat_bias(t['cb'], NDA))
            boc_s = cp.tile([128, NDA], F32)
            nc.scalar.dma_start(out=boc_s, in_=_feat_bias(t['boc'], NDA))
            # tail weights after conv weights on the same separate FIFO
            nc.scalar.dma_start(out=woa_s, in_=_chunked(t['woaT'], NDA, DA))
            nc.scalar.dma_start(out=boa_s, in_=_feat_bias(t['boa'], NDA))
            nc.scalar.dma_start(out=wf_s, in_=_chunked(t['wfT'], ND, D))

            # effective conv bias: Wo_conv @ cb + boc (also PE warm-up)
            for oc in range(NDA):
                pcb = ps_cb.tile([128, 1], F32, tag='cb')
                for c in range(NDA):
                    nc.tensor.matmul(pcb[:, :],
                                     wocT_s[:, c, oc * 128:(oc + 1) * 128],
                                     cb_s[:, c:c + 1],
                                     start=(c == 0), stop=(c == NDA - 1))
                nc.scalar.activation(cbe_s[:, oc:oc + 1], pcb[:, :],
                                     Identity, bias=boc_s[:, oc:oc + 1])

            for k in range(3):
                for c in range(NDA):
                    nc.vector.tensor_scalar_mul(
                        mc_s[:, k, c, :], wocT_s[:, c, :],
                        cw_s[:, c, k:k + 1])

            # ------------- Q/K/V projections (sync-queue DMAs) -------------
            with tc.tile_pool(name=R + 'projw', bufs=1) as wpj, \
                 tc.tile_pool(name=R + 'inp', bufs=1) as ip, \
                 tc.tile_pool(name=R + 'ps_proj', bufs=2,
                              space='PSUM') as ps_proj:
                wq_s = wpj.tile([128, NDA, DA], BF16, tag='wq')
                nc.sync.dma_start(out=wq_s, in_=_chunked(t['wqT'], NDA, DA))
                bq_s = wpj.tile([128, NDA], F32, tag='bq')
                nc.sync.dma_start(out=bq_s, in_=_feat_bias(t['bq'], NDA))
                qa_in, k_in, v_in = [], [], []
                for tt in range(NTQ):
                    x = ip.tile([128, NDA, 512], BF16, tag=f'qa{tt}')
                    nc.sync.dma_start(
                        out=x, in_=_chunked(t['qaT'], NDA, TQ, tt * 512, 512))
                    qa_in.append(x)
                wk_s = wpj.tile([128, NDA, DA], BF16, tag='wk')
                nc.sync.dma_start(out=wk_s, in_=_chunked(t['wkT'], NDA, DA))
                bk_s = wpj.tile([128, NDA], F32, tag='bk')
                nc.sync.dma_start(out=bk_s, in_=_feat_bias(t['bk'], NDA))
                for tt in range(NTK):
                    x = ip.tile([128, NDA, 512], BF16, tag=f'ki{tt}')
                    nc.sync.dma_start(
                        out=x, in_=_chunked(t['kT'], NDA, TK, tt * 512, 512))
                    k_in.append(x)
                wv_s = wpj.tile([128, NDA, DA], BF16, tag='wv')
                nc.sync.dma_start(out=wv_s, in_=_chunked(t['wvT'], NDA, DA))
                bv_bc = wpj.tile([128, DA], F32, tag='bvb')
                nc.sync.dma_start(out=bv_bc, in_=_bcast(t['bv'], DA))
                for tt in range(NTK):
                    x = ip.tile([128, NDA, 512], BF16, tag=f'vi{tt}')
                    nc.sync.dma_start(
                        out=x, in_=_chunked(t['vT'], NDA, TK, tt * 512, 512))
                    v_in.append(x)
                nc.sync.dma_start(out=qcT_s,
                                  in_=_chunked(t['qcT'], NDA, TQ + 2))
                nc.sync.dma_start(out=bf_bc, in_=_bcast(t['bf'], D))
                nc.vector.memset(vaug[:, :, :, 0:64], 1.0)

                # Q: feature-major out
                for tt in range(NTQ):
                    for oc in range(NDA):
                        pq = ps_proj.tile([128, 512], F32, tag='projp')
                        for dc in range(NDA):
                            nc.tensor.matmul(
                                pq[:, :],
                                wq_s[:, dc, oc * 128:(oc + 1) * 128],
                                qa_in[tt][:, dc, :],
                                start=(dc == 0), stop=(dc == NDA - 1))
                        nc.scalar.activation(
                            q_s[:, oc, tt * 512:(tt + 1) * 512], pq[:, :],
                            Identity, bias=bq_s[:, oc:oc + 1])

                # K: feature-major out
                for tt in range(NTK):
                    for oc in range(NDA):
                        pk = ps_proj.tile([128, 512], F32, tag='projp')
                        for dc in range(NDA):
                            nc.tensor.matmul(
                                pk[:, :],
                                wk_s[:, dc, oc * 128:(oc + 1) * 128],
                                k_in[tt][:, dc, :],
                                start=(dc == 0), stop=(dc == NDA - 1))
                        nc.scalar.activation(
                            k_s[:, oc, tt * 512:(tt + 1) * 512], pk[:, :],
                            Identity, bias=bk_s[:, oc:oc + 1])

                # V: token-major out into vaug[:, kc, h, 64:128]
                for kc in range(NKC):
                    pv = ps_proj.tile([128, 512], F32, tag='projp')
                    for dc in range(NDA):
                        nc.tensor.matmul(
                            pv[:, :],
                            v_in[kc // 4][:, dc, (kc % 4) * 128:
                                          (kc % 4 + 1) * 128],
                            wv_s[:, dc, :],
                            start=(dc == 0), stop=(dc == NDA - 1))
                    nc.vector.tensor_tensor(
                        vaug[:, kc, :, 64:128],
                        pv[:].rearrange('p (h x) -> p h x', h=H),
                        bv_bc[:].rearrange('p (h x) -> p h x', h=H),
                        AOp.add)

        # residual prefetch + bf fold (GpSimd, runs under attention)
        nc.sync.dma_start(out=resbf, in_=_chunked(t['qres'], NQC, D))
        nc.gpsimd.tensor_tensor(
            resbf[:, :, :], resbf[:, :, :],
            bass.AP(tensor=bf_bc.tensor, offset=bf_bc.offset,
                    ap=[[bf_bc.ap[0][0], 128], [0, NQC], [1, D]]),
            AOp.add)

        # ---------------- attention (+ conv matmuls drip-fed) ----------------
        with tc.tile_pool(name=R + 'pst', bufs=2, space='PSUM') as ps_s, \
             tc.tile_pool(name=R + 'pso', bufs=1, space='PSUM') as ps_o, \
             tc.tile_pool(name=R + 'psc', bufs=2, space='PSUM') as ps_c, \
             tc.tile_pool(name=R + 'pwork', bufs=3) as wp, \
             tc.tile_pool(name=R + 'rnorm', bufs=2) as rp:

            # conv matmul drip generator: yields after each emitted matmul
            def conv_steps():
                for oc in range(NDA):
                    for tt in range(NTQ):
                        pc = ps_c.tile([128, 512], F32, tag='convp')
                        n = 0
                        for k in range(3):
                            for c in range(NDA):
                                nc.tensor.matmul(
                                    pc[:, :],
                                    mc_s[:, k, c, oc * 128:(oc + 1) * 128],
                                    qcT_s[:, c, tt * 512 + k: tt * 512 + k + 512],
                                    start=(n == 0), stop=(n == 3 * NDA - 1),
                                    skip_group_check=True)
                                n += 1
                                yield
                        nc.vector.tensor_scalar_add(
                            fT_s[:, NDA + oc, tt * 512:(tt + 1) * 512],
                            pc[:, :], cbe_s[:, oc:oc + 1])

            conv_gen = conv_steps()

            def conv_drip():
                try:
                    next(conv_gen)
                except StopIteration:
                    pass

            for h in range(H):
                hp = (h % 2) * 64
                hc = h // 2
                o_ps = ps_o.tile([128, TQ], F32, tag='o')
                prev = None  # pending PV p_sb
                for kc in range(NKC):
                    s_ps = ps_s.tile([128, TQ], F32, tag='s')
                    for tt in range(NTQ):
                        nc.tensor.matmul(
                            s_ps[:, tt * 512:(tt + 1) * 512],
                            k_s[hp:hp + 64, hc, kc * 128:(kc + 1) * 128],
                            q_s[hp:hp + 64, hc, tt * 512:(tt + 1) * 512],
                            start=True, stop=True)
                    if prev is not None:
                        _emit_pv(nc, o_ps, vaug, prev[1], h, prev[0])
                        conv_drip()
                    p_sb = wp.tile([128, TQ], BF16, tag='p')
                    nc.scalar.activation(p_sb[:, :], s_ps[:, :], Exp,
                                         scale=0.125)
                    prev = (kc, p_sb)
                _emit_pv(nc, o_ps, vaug, prev[1], h, prev[0])
                conv_drip()

                # normalize: rows 0:64 = denominator (broadcast), 64:128 = O
                rec = rp.tile([64, TQ], F32, tag='rec')
                nc.vector.reciprocal_approx_fast(rec[:, :], o_ps[0:64, :])
                nc.vector.tensor_tensor(
                    o_s[hp:hp + 64, hc, :], o_ps[64:128, :], rec[:, :],
                    AOp.mult)

            for _ in range(3 * NDA * NDA * NTQ):
                conv_drip()

        # ---------------- Wo_attn ----------------
        with tc.tile_pool(name=R + 'ps_wo', bufs=2, space='PSUM') as ps_wo:
            for tt in range(NTQ):
                for oc in range(NDA):
                    pa = ps_wo.tile([128, 512], F32, tag='wop')
                    for dc in range(NDA):
                        nc.tensor.matmul(
                            pa[:, :],
                            woa_s[:, dc, oc * 128:(oc + 1) * 128],
                            o_s[:, dc, tt * 512:(tt + 1) * 512],
                            start=(dc == 0), stop=(dc == NDA - 1))
                    nc.scalar.activation(
                        fT_s[:, oc, tt * 512:(tt + 1) * 512], pa[:, :],
                        Identity, bias=boa_s[:, oc:oc + 1])

        # ---------------- Wf + residual + LayerNorm ----------------
        with tc.tile_pool(name=R + 'lnw', bufs=3) as lp, \
             tc.tile_pool(name=R + 'ps_f', bufs=2, space='PSUM') as ps_f:
            for i in range(NQC):
                pf = ps_f.tile([128, D], F32, tag='f')
                for ot in range(2):
                    for fc in range(ND):
                        nc.tensor.matmul(
                            pf[:, ot * 512:(ot + 1) * 512],
                            fT_s[:, fc, i * 128:(i + 1) * 128],
                            wf_s[:, fc, ot * 512:(ot + 1) * 512],
                            start=(fc == 0), stop=(fc == ND - 1))
                x_s = lp.tile([128, D], F32, tag='x')
                nc.vector.scalar_tensor_tensor(x_s[:, :], pf[:, :], 1.0,
                                               resbf[:, i, :], AOp.mult,
                                               AOp.add)
                stats = lp.tile([128, 2, 6], F32, tag='st')
                nc.vector.bn_stats(stats[:, 0, :], x_s[:, 0:512])
                nc.vector.bn_stats(stats[:, 1, :], x_s[:, 512:1024])
                mv = lp.tile([128, 2], F32, tag='mv')
                nc.vector.bn_aggr(mv[:, :], stats[:, :, :])
                sd = lp.tile([128, 1], F32, tag='sd')
                nc.scalar.activation(sd[:, :], mv[:, 1:2], Sqrt,
                                     bias=eps_s[:, 0:1])
                rstd = lp.tile([128, 1], F32, tag='rs')
                nc.vector.reciprocal(rstd[:, :], sd[:, :])
                nm = lp.tile([128, 1], F32, tag='nm')
                nc.vector.scalar_tensor_tensor(nm[:, :], mv[:, 0:1], -1.0,
                                               rstd[:, :], AOp.mult, AOp.mult)
                t1 = lp.tile([128, D], BF16, tag='t1')
                nc.scalar.activation(t1[:, :], x_s[:, :], Identity,
                                     bias=nm[:, 0:1], scale=rstd[:, 0:1])
                o1 = lp.tile([128, D], BF16, tag='o1')
                nc.vector.tensor_tensor(o1[:, :], t1[:, :], ga_bc[:, :],
                                        AOp.mult)
                o_sb = lp.tile([128, D], BF16, tag='ob')
                nc.gpsimd.tensor_tensor(o_sb[:, :], o1[:, :], be_bc[:, :],
                                        AOp.add)
                nc.gpsimd.dma_start(out=_rows(t['out'], D, i * 128, 128),
                                    in_=o_sb[:, :])


def _emit_pv(nc, o_ps, vaug, p_sb, h, kc):
    for tt in range(NTQ):
        nc.tensor.matmul(
            o_ps[:, tt * 512:(tt + 1) * 512],
            vaug[:, kc, h, :],
            p_sb[:, tt * 512:(tt + 1) * 512],
            start=(kc == 0), stop=(kc == NKC - 1),
            skip_group_check=True)


def make_in_maps(inputs):
    q = np.ascontiguousarray(np.asarray(inputs['queries'], np.float32))
    k = np.ascontiguousarray(np.asarray(inputs['keys'], np.float32))
    v = np.ascontiguousarray(np.asarray(inputs['values'], np.float32))
    W = {n: np.ascontiguousarray(np.asarray(inputs[n], np.float32).T)
         for n in ('Wq', 'Wk', 'Wv', 'Wo_attn', 'Wo_conv', 'Wf')}
    com = {
        'wqT': W['Wq'].astype(BF), 'wkT': W['Wk'].astype(BF),
        'wvT': W['Wv'].astype(BF), 'woaT': W['Wo_attn'].astype(BF),
        'wocT': W['Wo_conv'], 'wfT': W['Wf'].astype(BF),
        'cw': np.asarray(inputs['conv_w'], np.float32).reshape(DC, 3),
        'bq': np.asarray(inputs['bq'], np.float32),
        'bk': np.asarray(inputs['bk'], np.float32),
        'bv': np.asarray(inputs['bv'], np.float32),
        'boa': np.asarray(inputs['bo_attn'], np.float32),
        'cb': np.asarray(inputs['conv_b'], np.float32),
        'boc': np.asarray(inputs['bo_conv'], np.float32),
        'bf': np.asarray(inputs['bf'], np.float32),
        'gamma': np.asarray(inputs['gamma'], np.float32),
        'beta': np.asarray(inputs['beta'], np.float32),
    }
    com = {n: np.ascontiguousarray(a) for n, a in com.items()}
    in_maps = []
    for core in range(N_CORES):
        b, half = core // 2, core % 2
        r0, r1 = half * TQ, (half + 1) * TQ
        qc = np.zeros((TQ + 2, DC), np.float32)
        qc[1:TQ + 1] = q[b, r0:r1, DA:]
        if r0 > 0:
            qc[0] = q[b, r0 - 1, DA:]
        if r1 < L:
            qc[TQ + 1] = q[b, r1, DA:]
        m = dict(com)
        m['qaT'] = np.ascontiguousarray(q[b, r0:r1, :DA].T).astype(BF)
        m['qcT'] = np.ascontiguousarray(qc.T).astype(BF)
        m['qres'] = np.ascontiguousarray(q[b, r0:r1, :]).astype(BF)
        m['kT'] = np.ascontiguousarray(k[b, :, :DA].T).astype(BF)
        m['vT'] = np.ascontiguousarray(v[b, :, :DA].T).astype(BF)
        in_maps.append(m)
    return in_maps


_NC_CACHE = {}


def get_nc(reps=1):
    if reps not in _NC_CACHE:
        _NC_CACHE[reps] = build_nc(reps)
    return _NC_CACHE[reps]


def kernel(**inputs):
    from concourse.bass_utils import run_bass_kernel_spmd
    nc = get_nc(1)
    in_maps = make_in_maps(inputs)
    res = run_bass_kernel_spmd(nc, in_maps, core_ids=list(range(N_CORES)))
    out = np.empty((B, L, D), np.float32)
    for core in range(N_CORES):
        b, half = core // 2, core % 2
        out[b, half * TQ:(half + 1) * TQ, :] = res.results[core]['out']
    return out


# revision 17
# speedup vs baseline: 1.0585x; 1.0026x over previous
"""CSPAttention Trainium2 kernel: 8-way SPMD (batch x seq-half), no collectives.

Sharding: core = b*2 + half; each core computes 1024 query rows of batch b
against the full 2048-token K/V of that batch.  Host side does layout
marshalling only (transposes + fp32->bf16 casts); all FLOPs run on device.

Device plan (per core, bf16 matmuls, fp32 PSUM accumulate):
  qkv:    feature-major Q/K (features on partitions, tokens free) via
          weight-stationary matmuls; V token-major into the PV stationary
          layout vaug[kv, kc, h, ones(64)|V(64)] so the softmax denominator
          falls out of the PV matmul broadcast across 64 PSUM partitions.
  attn:   per head, per kv-chunk kc: S.T = K_chunk.T @ Q (2 matmuls into one
          2-bank PSUM tile), one Exp ACTIVATE over [128,1024] (scale=1/8
          folded) -> bf16 SBUF, PV accumulates [denom|O] over kc.  The PE
          instruction stream is software-pipelined (S of chunk kc+1 is
          emitted before PV of chunk kc) so the PE never queue-blocks on the
          ACT exp; conv-branch matmuls are drip-fed into the same loop to
          fill leftover PE slack (the loop is ACT-bound).
  norm:   reciprocal_approx_fast on the denominator rows + DVE multiply.
  tail:   Wo_attn, then Wf token-major + residual + LayerNorm, spread over
          DVE (stats) / ACT (scale-bias apply) / GpSimd (residual prep, out
          DMA with bf16->f32 cast) so no single engine serializes the tail.
"""

import sys

sys.path.insert(0, '/opt/trn_rl_repo')

import numpy as np
import ml_dtypes

import concourse.bass as bass
import concourse.tile as tile
from concourse import bacc, mybir

F32 = mybir.dt.float32
BF16 = mybir.dt.bfloat16
BF = ml_dtypes.bfloat16

B, L, D = 4, 2048, 1024
DA = 512          # attention channels
DC = 512          # conv channels
H = 8             # heads
HD = 64           # head dim
N_CORES = 8
TQ = 1024         # query rows per core
TK = 2048         # kv rows per core
NTQ = TQ // 512   # 512-token tiles of queries
NTK = TK // 512
NQC = TQ // 128   # 128-token query chunks
NKC = TK // 128   # 128-token kv chunks
NDA = DA // 128
ND = D // 128
LN_EPS = 1e-5

Identity = mybir.ActivationFunctionType.Identity
Exp = mybir.ActivationFunctionType.Exp
Sqrt = mybir.ActivationFunctionType.Sqrt
AOp = mybir.AluOpType


def _chunked(t, nch, w, col0=0, ncol=None):
    """DRAM [nch*128, w] -> SBUF-layout AP [128, nch, ncol] starting at col0."""
    if ncol is None:
        ncol = w
    return bass.AP(tensor=t, offset=col0,
                   ap=[[w, 128], [128 * w, nch], [1, ncol]])


def _feat_bias(t, nch):
    """DRAM [nch*128] -> SBUF [128, nch] feature-major bias."""
    return bass.AP(tensor=t, offset=0, ap=[[1, 128], [128, nch]])


def _bcast(t, n):
    """DRAM [n] -> [128, n] partition broadcast."""
    return bass.AP(tensor=t, offset=0, ap=[[0, 128], [1, n]])


def _rows(t, w, r0, nr):
    """DRAM [*, w] rows r0:r0+nr -> SBUF [nr, w]."""
    return bass.AP(tensor=t, offset=r0 * w, ap=[[w, nr], [1, w]])


def build_nc(reps: int = 1):
    nc = bacc.Bacc('TRN2', target_bir_lowering=False, debug=False,
                   num_devices=N_CORES)

    def din(name, shape, dt):
        return nc.dram_tensor(name, list(shape), dt, kind='ExternalInput')

    t = {n: din(n, s, dt) for n, s, dt in [
        ('qaT', [DA, TQ], BF16), ('qcT', [DC, TQ + 2], BF16),
        ('qres', [TQ, D], BF16),
        ('kT', [DA, TK], BF16), ('vT', [DA, TK], BF16),
        ('wqT', [DA, DA], BF16), ('wkT', [DA, DA], BF16),
        ('wvT', [DA, DA], BF16), ('woaT', [DA, DA], BF16),
        ('wocT', [DC, DC], F32), ('wfT', [D, D], BF16),
        ('cw', [DC, 3], F32), ('bq', [DA], F32), ('bk', [DA], F32),
        ('bv', [DA], F32), ('boa', [DA], F32), ('cb', [DC], F32),
        ('boc', [DC], F32), ('bf', [D], F32), ('gamma', [D], F32),
        ('beta', [D], F32)]}
    t['out'] = nc.dram_tensor('out', [TQ, D], F32, kind='ExternalOutput')

    with tile.TileContext(nc) as tc:
        for rep in range(reps):
            _build_rep(nc, tc, rep, t)
    nc.compile()
    return nc


def _build_rep(nc, tc, rep, t):
    R = f'r{rep}_'
    with tc.tile_pool(name=R + 'persist', bufs=1) as pp:
        fT_s = pp.tile([128, ND, TQ], BF16)       # concat features, fm
        q_s = pp.tile([128, NDA, TQ], BF16)
        k_s = pp.tile([128, NDA, TK], BF16)
        vaug = pp.tile([128, NKC, H, 128], BF16)  # [ones(64) | V(64)]
        o_s = pp.tile([128, NDA, TQ], BF16)       # normalized attn out, fm
        qcT_s = pp.tile([128, NDA, TQ + 2], BF16)
        mc_s = pp.tile([128, 3, NDA, DC], BF16)   # conv folded weights
        cbe_s = pp.tile([128, NDA], F32)          # conv effective bias
        resbf = pp.tile([128, NQC, D], BF16)      # residual + bf, token-major
        bf_bc = pp.tile([128, D], F32)
        wf_s = pp.tile([128, ND, D], BF16)        # prefetched tail weights
        woa_s = pp.tile([128, NDA, DA], BF16)
        boa_s = pp.tile([128, NDA], F32)
        ga_bc = pp.tile([128, D], BF16)
        be_bc = pp.tile([128, D], BF16)
        eps_s = pp.tile([128, 1], F32)
        nc.vector.memset(eps_s, LN_EPS)
        nc.gpsimd.dma_start(out=ga_bc, in_=_bcast(t['gamma'], D))
        nc.gpsimd.dma_start(out=be_bc, in_=_bcast(t['beta'], D))

        # preload the exp table set while DMAs stream
        warm = pp.tile([128, 1], F32)
        nc.vector.memset(warm, 0.0)
        nc.scalar.activation(warm, warm, Exp)

        # ---------------- conv weight folding (ACT-queue DMAs) ------------
        with tc.tile_pool(name=R + 'convw', bufs=1) as cp, \
             tc.tile_pool(name=R + 'ps_cb', bufs=1, space='PSUM') as ps_cb:
            wocT_s = cp.tile([128, NDA, DC], F32)
            nc.scalar.dma_start(out=wocT_s, in_=_chunked(t['wocT'], NDA, DC))
            cw_s = cp.tile([128, NDA, 3], F32)
            nc.scalar.dma_start(out=cw_s, in_=_chunked(t['cw'], NDA, 3))
            cb_s = cp.tile([128, NDA], F32)
            nc.scalar.dma_start(out=cb_s, in_=_feat_bias(t['cb'], NDA))
            boc_s = cp.tile([128, NDA], F32)
            nc.scalar.dma_start(out=boc_s, in_=_feat_bias(t['boc'], NDA))
            # tail weights after conv weights on the same separate FIFO
            nc.scalar.dma_start(out=woa_s, in_=_chunked(t['woaT'], NDA, DA))
            nc.scalar.dma_start(out=boa_s, in_=_feat_bias(t['boa'], NDA))
            nc.scalar.dma_start(out=wf_s, in_=_chunked(t['wfT'], ND, D))

            # effective conv bias: Wo_conv @ cb + boc (also PE warm-up)
            for oc in range(NDA):
                pcb = ps_cb.tile([128, 1], F32, tag='cb')
                for c in range(NDA):
                    nc.tensor.matmul(pcb[:, :],
                                     wocT_s[:, c, oc * 128:(oc + 1) * 128],
                                     cb_s[:, c:c + 1],
                                     start=(c == 0), stop=(c == NDA - 1))
                nc.scalar.activation(cbe_s[:, oc:oc + 1], pcb[:, :],
                                     Identity, bias=boc_s[:, oc:oc + 1])

            for k in range(3):
                for c in range(NDA):
                    nc.vector.tensor_scalar_mul(
                        mc_s[:, k, c, :], wocT_s[:, c, :],
                        cw_s[:, c, k:k + 1])

            # ------------- Q/K/V projections (sync-queue DMAs) -------------
            with tc.tile_pool(name=R + 'projw', bufs=1) as wpj, \
                 tc.tile_pool(name=R + 'inp', bufs=1) as ip, \
                 tc.tile_pool(name=R + 'ps_proj', bufs=2,
                              space='PSUM') as ps_proj:
                wq_s = wpj.tile([128, NDA, DA], BF16, tag='wq')
                nc.sync.dma_start(out=wq_s, in_=_chunked(t['wqT'], NDA, DA))
                bq_s = wpj.tile([128, NDA], F32, tag='bq')
                nc.sync.dma_start(out=bq_s, in_=_feat_bias(t['bq'], NDA))
                qa_in, k_in, v_in = [], [], []
                for tt in range(NTQ):
                    x = ip.tile([128, NDA, 512], BF16, tag=f'qa{tt}')
                    nc.sync.dma_start(
                        out=x, in_=_chunked(t['qaT'], NDA, TQ, tt * 512, 512))
                    qa_in.append(x)
                wk_s = wpj.tile([128, NDA, DA], BF16, tag='wk')
                nc.sync.dma_start(out=wk_s, in_=_chunked(t['wkT'], NDA, DA))
                bk_s = wpj.tile([128, NDA], F32, tag='bk')
                nc.sync.dma_start(out=bk_s, in_=_feat_bias(t['bk'], NDA))
                for tt in range(NTK):
                    x = ip.tile([128, NDA, 512], BF16, tag=f'ki{tt}')
                    nc.sync.dma_start(
                        out=x, in_=_chunked(t['kT'], NDA, TK, tt * 512, 512))
                    k_in.append(x)
                wv_s = wpj.tile([128, NDA, DA], BF16, tag='wv')
                nc.sync.dma_start(out=wv_s, in_=_chunked(t['wvT'], NDA, DA))
                bv_bc = wpj.tile([128, DA], F32, tag='bvb')
                nc.sync.dma_start(out=bv_bc, in_=_bcast(t['bv'], DA))
                for tt in range(NTK):
                    x = ip.tile([128, NDA, 512], BF16, tag=f'vi{tt}')
                    nc.sync.dma_start(
                        out=x, in_=_chunked(t['vT'], NDA, TK, tt * 512, 512))
                    v_in.append(x)
                nc.sync.dma_start(out=qcT_s,
                                  in_=_chunked(t['qcT'], NDA, TQ + 2))
                nc.sync.dma_start(out=bf_bc, in_=_bcast(t['bf'], D))
                nc.vector.memset(vaug[:, :, :, 0:64], 1.0)

                # Q: feature-major out; stationary reused across both q tiles
                for oc in range(NDA):
                    pq = [ps_proj.tile([128, 512], F32, tag=f'projp{tt}',
                                       name=f'pq{tt}') for tt in range(NTQ)]
                    for dc in range(NDA):
                        for tt in range(NTQ):
                            nc.tensor.matmul(
                                pq[tt][:, :],
                                wq_s[:, dc, oc * 128:(oc + 1) * 128],
                                qa_in[tt][:, dc, :],
                                start=(dc == 0), stop=(dc == NDA - 1))
                    for tt in range(NTQ):
                        nc.scalar.activation(
                            q_s[:, oc, tt * 512:(tt + 1) * 512], pq[tt][:, :],
                            Identity, bias=bq_s[:, oc:oc + 1])

                # K: feature-major out; stationary reused across 2-tile pairs
                for tg in range(2):
                    for oc in range(NDA):
                        pk = [ps_proj.tile([128, 512], F32, tag=f'projp{tt}',
                                           name=f'pk{tt}') for tt in range(2)]
                        for dc in range(NDA):
                            for tt in range(2):
                                nc.tensor.matmul(
                                    pk[tt][:, :],
                                    wk_s[:, dc, oc * 128:(oc + 1) * 128],
                                    k_in[tg * 2 + tt][:, dc, :],
                                    start=(dc == 0), stop=(dc == NDA - 1))
                        for tt in range(2):
                            nc.scalar.activation(
                                k_s[:, oc, (tg * 2 + tt) * 512:
                                    (tg * 2 + tt + 1) * 512], pk[tt][:, :],
                                Identity, bias=bk_s[:, oc:oc + 1])

                # V: token-major out into vaug[:, kc, h, 64:128]
                for kc in range(NKC):
                    pv = ps_proj.tile([128, 512], F32, tag='projp')
                    for dc in range(NDA):
                        nc.tensor.matmul(
                            pv[:, :],
                            v_in[kc // 4][:, dc, (kc % 4) * 128:
                                          (kc % 4 + 1) * 128],
                            wv_s[:, dc, :],
                            start=(dc == 0), stop=(dc == NDA - 1))
                    nc.vector.tensor_tensor(
                        vaug[:, kc, :, 64:128],
                        pv[:].rearrange('p (h x) -> p h x', h=H),
                        bv_bc[:].rearrange('p (h x) -> p h x', h=H),
                        AOp.add)

        # residual prefetch + bf fold (GpSimd, runs under attention)
        nc.sync.dma_start(out=resbf, in_=_chunked(t['qres'], NQC, D))
        nc.gpsimd.tensor_tensor(
            resbf[:, :, :], resbf[:, :, :],
            bass.AP(tensor=bf_bc.tensor, offset=bf_bc.offset,
                    ap=[[bf_bc.ap[0][0], 128], [0, NQC], [1, D]]),
            AOp.add)

        # ---------------- attention (+ conv matmuls drip-fed) ----------------
        with tc.tile_pool(name=R + 'pst', bufs=2, space='PSUM') as ps_s, \
             tc.tile_pool(name=R + 'pso', bufs=1, space='PSUM') as ps_o, \
             tc.tile_pool(name=R + 'psc', bufs=2, space='PSUM') as ps_c, \
             tc.tile_pool(name=R + 'pwork', bufs=3) as wp, \
             tc.tile_pool(name=R + 'rnorm', bufs=2) as rp:

            # conv matmul drip generator: yields after each emitted matmul
            def conv_steps():
                for oc in range(NDA):
                    for tt in range(NTQ):
                        pc = ps_c.tile([128, 512], F32, tag='convp')
                        n = 0
                        for k in range(3):
                            for c in range(NDA):
                                nc.tensor.matmul(
                                    pc[:, :],
                                    mc_s[:, k, c, oc * 128:(oc + 1) * 128],
                                    qcT_s[:, c, tt * 512 + k: tt * 512 + k + 512],
                                    start=(n == 0), stop=(n == 3 * NDA - 1),
                                    skip_group_check=True)
                                n += 1
                                yield
                        nc.vector.tensor_scalar_add(
                            fT_s[:, NDA + oc, tt * 512:(tt + 1) * 512],
                            pc[:, :], cbe_s[:, oc:oc + 1])

            conv_gen = conv_steps()

            def conv_drip():
                try:
                    next(conv_gen)
                except StopIteration:
                    pass

            for h in range(H):
                hp = (h % 2) * 64
                hc = h // 2
                o_ps = ps_o.tile([128, TQ], F32, tag='o')
                prev = None  # pending PV p_sb
                for kc in range(NKC):
                    s_ps = ps_s.tile([128, TQ], F32, tag='s')
                    for tt in range(NTQ):
                        nc.tensor.matmul(
                            s_ps[:, tt * 512:(tt + 1) * 512],
                            k_s[hp:hp + 64, hc, kc * 128:(kc + 1) * 128],
                            q_s[hp:hp + 64, hc, tt * 512:(tt + 1) * 512],
                            start=True, stop=True)
                    if prev is not None:
                        _emit_pv(nc, o_ps, vaug, prev[1], h, prev[0])
                        conv_drip()
                    p_sb = wp.tile([128, TQ], BF16, tag='p')
                    nc.scalar.activation(p_sb[:, :], s_ps[:, :], Exp,
                                         scale=0.125)
                    prev = (kc, p_sb)
                _emit_pv(nc, o_ps, vaug, prev[1], h, prev[0])
                conv_drip()

                # normalize: rows 0:64 = denominator (broadcast), 64:128 = O
                rec = rp.tile([64, TQ], F32, tag='rec')
                nc.vector.reciprocal_approx_fast(rec[:, :], o_ps[0:64, :])
                nc.vector.tensor_tensor(
                    o_s[hp:hp + 64, hc, :], o_ps[64:128, :], rec[:, :],
                    AOp.mult)

            for _ in range(3 * NDA * NDA * NTQ):
                conv_drip()

        # ---------------- Wo_attn (stationary reused across q tiles) -------
        with tc.tile_pool(name=R + 'ps_wo', bufs=2, space='PSUM') as ps_wo:
            for oc in range(NDA):
                pa = [ps_wo.tile([128, 512], F32, tag=f'wop{tt}',
                                 name=f'pa{tt}') for tt in range(NTQ)]
                for dc in range(NDA):
                    for tt in range(NTQ):
                        nc.tensor.matmul(
                            pa[tt][:, :],
                            woa_s[:, dc, oc * 128:(oc + 1) * 128],
                            o_s[:, dc, tt * 512:(tt + 1) * 512],
                            start=(dc == 0), stop=(dc == NDA - 1))
                for tt in range(NTQ):
                    nc.scalar.activation(
                        fT_s[:, oc, tt * 512:(tt + 1) * 512], pa[tt][:, :],
                        Identity, bias=boa_s[:, oc:oc + 1])

        # ---------------- Wf + residual + LayerNorm ----------------
        with tc.tile_pool(name=R + 'lnw', bufs=3) as lp, \
             tc.tile_pool(name=R + 'ps_f', bufs=2, space='PSUM') as ps_f:
            for i in range(NQC):
                pf = ps_f.tile([128, D], F32, tag='f')
                for fc in range(ND):
                    for ot in range(2):
                        nc.tensor.matmul(
                            pf[:, ot * 512:(ot + 1) * 512],
                            fT_s[:, fc, i * 128:(i + 1) * 128],
                            wf_s[:, fc, ot * 512:(ot + 1) * 512],
                            start=(fc == 0), stop=(fc == ND - 1))
                x_s = lp.tile([128, D], F32, tag='x')
                nc.vector.scalar_tensor_tensor(x_s[:, :], pf[:, :], 1.0,
                                               resbf[:, i, :], AOp.mult,
                                               AOp.add)
                stats = lp.tile([128, 2, 6], F32, tag='st')
                nc.vector.bn_stats(stats[:, 0, :], x_s[:, 0:512])
                nc.vector.bn_stats(stats[:, 1, :], x_s[:, 512:1024])
                mv = lp.tile([128, 2], F32, tag='mv')
                nc.vector.bn_aggr(mv[:, :], stats[:, :, :])
                sd = lp.tile([128, 1], F32, tag='sd')
                nc.scalar.activation(sd[:, :], mv[:, 1:2], Sqrt,
                                     bias=eps_s[:, 0:1])
                rstd = lp.tile([128, 1], F32, tag='rs')
                nc.vector.reciprocal(rstd[:, :], sd[:, :])
                nm = lp.tile([128, 1], F32, tag='nm')
                nc.vector.scalar_tensor_tensor(nm[:, :], mv[:, 0:1], -1.0,
                                               rstd[:, :], AOp.mult, AOp.mult)
                t1 = lp.tile([128, D], BF16, tag='t1')
                nc.scalar.activation(t1[:, :], x_s[:, :], Identity,
                                     bias=nm[:, 0:1], scale=rstd[:, 0:1])
                o1 = lp.tile([128, D], BF16, tag='o1')
                nc.vector.tensor_tensor(o1[:, :], t1[:, :], ga_bc[:, :],
                                        AOp.mult)
                o_sb = lp.tile([128, D], BF16, tag='ob')
                nc.gpsimd.tensor_tensor(o_sb[:, :], o1[:, :], be_bc[:, :],
                                        AOp.add)
                nc.gpsimd.dma_start(out=_rows(t['out'], D, i * 128, 128),
                                    in_=o_sb[:, :])


def _emit_pv(nc, o_ps, vaug, p_sb, h, kc):
    for tt in range(NTQ):
        nc.tensor.matmul(
            o_ps[:, tt * 512:(tt + 1) * 512],
            vaug[:, kc, h, :],
            p_sb[:, tt * 512:(tt + 1) * 512],
            start=(kc == 0), stop=(kc == NKC - 1),
            skip_group_check=True)


def make_in_maps(inputs):
    q = np.ascontiguousarray(np.asarray(inputs['queries'], np.float32))
    k = np.ascontiguousarray(np.asarray(inputs['keys'], np.float32))
    v = np.ascontiguousarray(np.asarray(inputs['values'], np.float32))
    W = {n: np.ascontiguousarray(np.asarray(inputs[n], np.float32).T)
         for n in ('Wq', 'Wk', 'Wv', 'Wo_attn', 'Wo_conv', 'Wf')}
    com = {
        'wqT': W['Wq'].astype(BF), 'wkT': W['Wk'].astype(BF),
        'wvT': W['Wv'].astype(BF), 'woaT': W['Wo_attn'].astype(BF),
        'wocT': W['Wo_conv'], 'wfT': W['Wf'].astype(BF),
        'cw': np.asarray(inputs['conv_w'], np.float32).reshape(DC, 3),
        'bq': np.asarray(inputs['bq'], np.float32),
        'bk': np.asarray(inputs['bk'], np.float32),
        'bv': np.asarray(inputs['bv'], np.float32),
        'boa': np.asarray(inputs['bo_attn'], np.float32),
        'cb': np.asarray(inputs['conv_b'], np.float32),
        'boc': np.asarray(inputs['bo_conv'], np.float32),
        'bf': np.asarray(inputs['bf'], np.float32),
        'gamma': np.asarray(inputs['gamma'], np.float32),
        'beta': np.asarray(inputs['beta'], np.float32),
    }
    com = {n: np.ascontiguousarray(a) for n, a in com.items()}
    in_maps = []
    for core in range(N_CORES):
        b, half = core // 2, core % 2
        r0, r1 = half * TQ, (half + 1) * TQ
        qc = np.zeros((TQ + 2, DC), np.float32)
        qc[1:TQ + 1] = q[b, r0:r1, DA:]
        if r0 > 0:
            qc[0] = q[b, r0 - 1, DA:]
        if r1 < L:
            qc[TQ + 1] = q[b, r1, DA:]
        m = dict(com)
        m['qaT'] = np.ascontiguousarray(q[b, r0:r1, :DA].T).astype(BF)
        m['qcT'] = np.ascontiguousarray(qc.T).astype(BF)
        m['qres'] = np.ascontiguousarray(q[b, r0:r1, :]).astype(BF)
        m['kT'] = np.ascontiguousarray(k[b, :, :DA].T).astype(BF)
        m['vT'] = np.ascontiguousarray(v[b, :, :DA].T).astype(BF)
        in_maps.append(m)
    return in_maps


_NC_CACHE = {}


def get_nc(reps=1):
    if reps not in _NC_CACHE:
        _NC_CACHE[reps] = build_nc(reps)
    return _NC_CACHE[reps]


def kernel(**inputs):
    from concourse.bass_utils import run_bass_kernel_spmd
    nc = get_nc(1)
    in_maps = make_in_maps(inputs)
    res = run_bass_kernel_spmd(nc, in_maps, core_ids=list(range(N_CORES)))
    out = np.empty((B, L, D), np.float32)
    for core in range(N_CORES):
        b, half = core // 2, core % 2
        out[b, half * TQ:(half + 1) * TQ, :] = res.results[core]['out']
    return out
